# revision 1
# baseline (speedup 1.0000x reference)
"""Distributed AttentionModel kernel for 8 trn2 NeuronCores.

Strategy: row-shard the 256x256 spatial grid across 8 cores (32 owned rows
per core) with a 17-row halo on each side (= receptive field of the whole
net up to the final conv). Each core recomputes its halo locally (no halo
exchanges); the only cross-core traffic is the per-channel InstanceNorm
statistics (sum / sum-of-squares), reduced with lax.psum. Each core emits
its 32 owned output rows; the host concatenates them.
"""
import numpy as np
import jax
import jax.numpy as jnp

L = 256
NDEV = 8
OWN = L // NDEV          # 32 rows owned per core
HALO = 17                # receptive field: 3 blocks (2*1+2*2+2*4) + f1 + attn + f2
SLAB = OWN + 2 * HALO    # 66 rows per core
DILATIONS = (1, 2, 4)
REGION = 3
EPS = 1e-5

SLAB_STARTS = [min(max(OWN * d - HALO, 0), L - SLAB) for d in range(NDEV)]
OWN_OFFS = [OWN * d - SLAB_STARTS[d] for d in range(NDEV)]


def _conv2d(x, w, b, dilation=1):
    k = w.shape[-1]
    p = dilation * (k - 1) // 2
    y = jax.lax.conv_general_dilated(
        x, w, window_strides=(1, 1), padding=((p, p), (p, p)),
        rhs_dilation=(dilation, dilation),
        dimension_numbers=('NCHW', 'OIHW', 'NCHW'))
    return y + b[None, :, None, None]


def _inorm(x, mask):
    # x: [1,C,S,256]; mask: [S] (1.0 on this core's owned rows).
    # Global mean/var over the full 256x256 image via psum of masked sums.
    xm = x * mask[None, None, :, None]
    s = jax.lax.psum(jnp.sum(xm, axis=(2, 3)), 'i')
    s2 = jax.lax.psum(jnp.sum(xm * x, axis=(2, 3)), 'i')
    n = float(L * L)
    m = s / n
    v = s2 / n - m * m
    return (x - m[:, :, None, None]) / jnp.sqrt(v[:, :, None, None] + EPS)


def _block(h, w1, b1, w2, b2, dilation, mask):
    y = jax.nn.relu(_inorm(_conv2d(h, w1, b1, dilation), mask))
    y = _inorm(_conv2d(y, w2, b2, dilation), mask)
    return jax.nn.relu(y + h)


def _regional_attention(x, wq, wk, wv, wo, bo):
    xh = x[0].transpose(1, 2, 0)            # [S,256,64]
    S, W = xh.shape[0], xh.shape[1]

    def heads(y):
        return y.reshape(S, W, 4, 16).transpose(2, 0, 1, 3)

    q = heads(xh @ wq.T)
    k = heads(xh @ wk.T)
    v = heads(xh @ wv.T)

    def gather(z):
        zp = jnp.pad(z, ((0, 0), (1, 1), (1, 1), (0, 0)))
        return jnp.stack([zp[:, i:i + S, j:j + W, :]
                          for i in range(REGION) for j in range(REGION)],
                         axis=3)            # [4,S,W,9,16]

    kn = gather(k)
    vn = gather(v)
    qk = jnp.einsum('hijc,hijnc->hijn', q, kn) / np.sqrt(64.0)
    attn = jax.nn.softmax(qk, axis=-1)
    out = jnp.einsum('hijn,hijnc->hijc', attn, vn)
    out = out.transpose(1, 2, 0, 3).reshape(1, S, W, 64)
    out = out @ wo.T + bo
    return out.transpose(0, 3, 1, 2)


def _forward(xs, params, mask, off):
    # xs: [441,SLAB,256] slab for this core; mask: [SLAB]; off: [] int32
    h = xs[None]
    h = jax.nn.relu(_inorm(_conv2d(h, params['c1_w'], params['c1_b']), mask))
    for i, dil in enumerate(DILATIONS):
        h = _block(h, params[f'b{i}_w1'], params[f'b{i}_b1'],
                   params[f'b{i}_w2'], params[f'b{i}_b2'], dil, mask)
    h = jax.nn.relu(_inorm(_conv2d(h, params['f1_w'], params['f1_b']), mask))
    h = _regional_attention(h, params['wq'], params['wk'], params['wv'],
                            params['wo'], params['bo'])
    y = _conv2d(h, params['f2_w'], params['f2_b'])   # [1,10,SLAB,256]
    return jax.lax.dynamic_slice_in_dim(y, off, OWN, axis=2)


_pmapped = jax.pmap(_forward, axis_name='i', in_axes=(0, None, 0, 0))


def kernel(x, params):
    x = np.asarray(x)
    slabs = np.stack([x[0, :, s:s + SLAB, :] for s in SLAB_STARTS])
    masks = np.zeros((NDEV, SLAB), dtype=np.float32)
    for d in range(NDEV):
        masks[d, OWN_OFFS[d]:OWN_OFFS[d] + OWN] = 1.0
    offs = np.asarray(OWN_OFFS, dtype=np.int32)
    params = {k: jnp.asarray(v) for k, v in params.items()}
    out = _pmapped(jnp.asarray(slabs), params, jnp.asarray(masks), offs)
    out = np.asarray(out)                    # [8,1,10,32,256]
    return out.transpose(1, 2, 0, 3, 4).reshape(1, 10, L, L)


# revision 2
# speedup vs baseline: 1.1253x; 1.1253x over previous
"""Distributed AttentionModel kernel for 8 trn2 NeuronCores.

Strategy: row-shard the 256x256 spatial grid across 8 cores (32 owned rows
per core) with a 17-row halo on each side (= receptive field of the whole
net up to the final conv). Each core recomputes its halo locally (no halo
exchanges); the only cross-core traffic is the per-channel InstanceNorm
statistics (sum / sum-of-squares), reduced with lax.psum. Each core emits
its 32 owned output rows; the host concatenates them.
"""
import numpy as np
import jax
import jax.numpy as jnp

L = 256
NDEV = 8
OWN = L // NDEV          # 32 rows owned per core
HALO = 17                # receptive field: 3 blocks (2*1+2*2+2*4) + f1 + attn + f2
SLAB = OWN + 2 * HALO    # 66 rows per core
DILATIONS = (1, 2, 4)
REGION = 3
EPS = 1e-5

SLAB_STARTS = [min(max(OWN * d - HALO, 0), L - SLAB) for d in range(NDEV)]
OWN_OFFS = [OWN * d - SLAB_STARTS[d] for d in range(NDEV)]


def _conv2d(x, w, b, dilation=1):
    k = w.shape[-1]
    p = dilation * (k - 1) // 2
    y = jax.lax.conv_general_dilated(
        x, w, window_strides=(1, 1), padding=((p, p), (p, p)),
        rhs_dilation=(dilation, dilation),
        dimension_numbers=('NCHW', 'OIHW', 'NCHW'))
    return y + b[None, :, None, None]


def _inorm(x, mask):
    # x: [1,C,S,256]; mask: [S] (1.0 on this core's owned rows).
    # Global mean/var over the full 256x256 image via psum of masked sums.
    xm = x * mask[None, None, :, None]
    s = jax.lax.psum(jnp.sum(xm, axis=(2, 3)), 'i')
    s2 = jax.lax.psum(jnp.sum(xm * x, axis=(2, 3)), 'i')
    n = float(L * L)
    m = s / n
    v = s2 / n - m * m
    return (x - m[:, :, None, None]) / jnp.sqrt(v[:, :, None, None] + EPS)


def _block(h, w1, b1, w2, b2, dilation, mask):
    y = jax.nn.relu(_inorm(_conv2d(h, w1, b1, dilation), mask))
    y = _inorm(_conv2d(y, w2, b2, dilation), mask)
    return jax.nn.relu(y + h)


def _regional_attention(x, wq, wk, wv, wo, bo):
    xh = x[0].transpose(1, 2, 0)            # [S,256,64]
    S, W = xh.shape[0], xh.shape[1]

    q = (xh @ wq.T).reshape(S, W, 4, 16)    # [S,W,h,c]
    k = (xh @ wk.T).reshape(S, W, 4, 16)
    v = (xh @ wv.T).reshape(S, W, 4, 16)

    kp = jnp.pad(k, ((1, 1), (1, 1), (0, 0), (0, 0)))
    vp = jnp.pad(v, ((1, 1), (1, 1), (0, 0), (0, 0)))
    shifts = [(i, j) for i in range(REGION) for j in range(REGION)]

    # logits per neighbor, accumulated without the [S,W,9,16] intermediate
    qs = q / np.sqrt(64.0)
    logits = [jnp.sum(qs * kp[i:i + S, j:j + W], axis=-1) for i, j in shifts]
    qk = jnp.stack(logits, axis=-1)         # [S,W,4,9]
    mx = jnp.max(qk, axis=-1, keepdims=True)
    e = jnp.exp(qk - mx)
    attn = e / jnp.sum(e, axis=-1, keepdims=True)

    out = jnp.zeros((S, W, 4, 16), dtype=x.dtype)
    for n, (i, j) in enumerate(shifts):
        out = out + attn[..., n:n + 1] * vp[i:i + S, j:j + W]
    out = out.reshape(S, W, 64) @ wo.T + bo
    return out.transpose(2, 0, 1)[None]


def _forward(xs, params, mask, off):
    # xs: [441,SLAB,256] slab for this core; mask: [SLAB]; off: [] int32
    h = xs[None]
    h = jax.nn.relu(_inorm(_conv2d(h, params['c1_w'], params['c1_b']), mask))
    for i, dil in enumerate(DILATIONS):
        h = _block(h, params[f'b{i}_w1'], params[f'b{i}_b1'],
                   params[f'b{i}_w2'], params[f'b{i}_b2'], dil, mask)
    h = jax.nn.relu(_inorm(_conv2d(h, params['f1_w'], params['f1_b']), mask))
    h = _regional_attention(h, params['wq'], params['wk'], params['wv'],
                            params['wo'], params['bo'])
    y = _conv2d(h, params['f2_w'], params['f2_b'])   # [1,10,SLAB,256]
    return jax.lax.dynamic_slice_in_dim(y, off, OWN, axis=2)


_pmapped = jax.pmap(_forward, axis_name='i', in_axes=(0, None, 0, 0))


def kernel(x, params):
    x = np.asarray(x)
    slabs = np.stack([x[0, :, s:s + SLAB, :] for s in SLAB_STARTS])
    masks = np.zeros((NDEV, SLAB), dtype=np.float32)
    for d in range(NDEV):
        masks[d, OWN_OFFS[d]:OWN_OFFS[d] + OWN] = 1.0
    offs = np.asarray(OWN_OFFS, dtype=np.int32)
    params = {k: jnp.asarray(v) for k, v in params.items()}
    out = _pmapped(jnp.asarray(slabs), params, jnp.asarray(masks), offs)
    out = np.asarray(out)                    # [8,1,10,32,256]
    return out.transpose(1, 2, 0, 3, 4).reshape(1, 10, L, L)


# revision 3
# speedup vs baseline: 1.1308x; 1.0049x over previous
"""Distributed AttentionModel kernel for 8 trn2 NeuronCores.

Strategy: row-shard the 256x256 spatial grid across 8 cores (32 owned rows
per core) with a 17-row halo on each side (= receptive field of the whole
net up to the final conv). Each core recomputes its halo locally (no halo
exchanges); the only cross-core traffic is the per-channel InstanceNorm
statistics (sum / sum-of-squares), reduced with lax.psum. Each core emits
its 32 owned output rows; the host concatenates them.
"""
import numpy as np
import jax
import jax.numpy as jnp

L = 256
NDEV = 8
OWN = L // NDEV          # 32 rows owned per core
HALO = 17                # receptive field: 3 blocks (2*1+2*2+2*4) + f1 + attn + f2
SLAB = OWN + 2 * HALO    # 66 rows per core
DILATIONS = (1, 2, 4)
REGION = 3
EPS = 1e-5

SLAB_STARTS = [min(max(OWN * d - HALO, 0), L - SLAB) for d in range(NDEV)]
OWN_OFFS = [OWN * d - SLAB_STARTS[d] for d in range(NDEV)]


def _conv2d(x, w, b, dilation=1):
    k = w.shape[-1]
    p = dilation * (k - 1) // 2
    y = jax.lax.conv_general_dilated(
        x, w, window_strides=(1, 1), padding=((p, p), (p, p)),
        rhs_dilation=(dilation, dilation),
        dimension_numbers=('NCHW', 'OIHW', 'NCHW'))
    return y + b[None, :, None, None]


def _inorm(x, mask):
    # x: [1,C,S,256]; mask: [S] (1.0 on this core's owned rows).
    # Global mean/var over the full 256x256 image via psum of masked sums.
    xm = x * mask[None, None, :, None]
    s = jax.lax.psum(jnp.sum(xm, axis=(2, 3)), 'i')
    s2 = jax.lax.psum(jnp.sum(xm * x, axis=(2, 3)), 'i')
    n = float(L * L)
    m = s / n
    v = s2 / n - m * m
    return (x - m[:, :, None, None]) / jnp.sqrt(v[:, :, None, None] + EPS)


def _block(h, w1, b1, w2, b2, dilation, mask):
    y = jax.nn.relu(_inorm(_conv2d(h, w1, b1, dilation), mask))
    y = _inorm(_conv2d(y, w2, b2, dilation), mask)
    return jax.nn.relu(y + h)


def _regional_attention(x, wq, wk, wv, wo, bo):
    xh = x[0].transpose(1, 2, 0)            # [S,256,64]
    S, W = xh.shape[0], xh.shape[1]

    q = (xh @ wq.T).reshape(S, W, 4, 16)    # [S,W,h,c]
    k = (xh @ wk.T).reshape(S, W, 4, 16)
    v = (xh @ wv.T).reshape(S, W, 4, 16)

    kp = jnp.pad(k, ((1, 1), (1, 1), (0, 0), (0, 0)))
    vp = jnp.pad(v, ((1, 1), (1, 1), (0, 0), (0, 0)))
    shifts = [(i, j) for i in range(REGION) for j in range(REGION)]

    # logits per neighbor, accumulated without the [S,W,9,16] intermediate
    qs = q / np.sqrt(64.0)
    logits = [jnp.sum(qs * kp[i:i + S, j:j + W], axis=-1) for i, j in shifts]
    qk = jnp.stack(logits, axis=-1)         # [S,W,4,9]
    mx = jnp.max(qk, axis=-1, keepdims=True)
    e = jnp.exp(qk - mx)
    attn = e / jnp.sum(e, axis=-1, keepdims=True)

    out = jnp.zeros((S, W, 4, 16), dtype=x.dtype)
    for n, (i, j) in enumerate(shifts):
        out = out + attn[..., n:n + 1] * vp[i:i + S, j:j + W]
    out = out.reshape(S, W, 64) @ wo.T + bo
    return out.transpose(2, 0, 1)[None]


def _forward(xs, params, mask, off):
    # xs: [441,SLAB,256] slab for this core; mask: [SLAB]; off: [] int32
    h = xs[None]
    h = jax.nn.relu(_inorm(_conv2d(h, params['c1_w'], params['c1_b']), mask))
    for i, dil in enumerate(DILATIONS):
        h = _block(h, params[f'b{i}_w1'], params[f'b{i}_b1'],
                   params[f'b{i}_w2'], params[f'b{i}_b2'], dil, mask)
    h = jax.nn.relu(_inorm(_conv2d(h, params['f1_w'], params['f1_b']), mask))
    h = _regional_attention(h, params['wq'], params['wk'], params['wv'],
                            params['wo'], params['bo'])
    y = _conv2d(h, params['f2_w'], params['f2_b'])   # [1,10,SLAB,256]
    return jax.lax.dynamic_slice_in_dim(y, off, OWN, axis=2)


_pmapped = jax.pmap(_forward, axis_name='i', in_axes=(0, None, 0, 0))


_cache = {}


def kernel(x, params):
    x = np.asarray(x)
    slabs = np.stack([x[0, :, s:s + SLAB, :] for s in SLAB_STARTS])
    if 'masks' not in _cache:
        masks = np.zeros((NDEV, SLAB), dtype=np.float32)
        for d in range(NDEV):
            masks[d, OWN_OFFS[d]:OWN_OFFS[d] + OWN] = 1.0
        _cache['masks'] = jnp.asarray(masks)
        _cache['offs'] = jnp.asarray(np.asarray(OWN_OFFS, dtype=np.int32))
    _cache['params'] = {k: jnp.asarray(v) for k, v in params.items()}
    out = _pmapped(jnp.asarray(slabs), _cache['params'],
                   _cache['masks'], _cache['offs'])
    out = np.asarray(out)                    # [8,1,10,32,256]
    return out.transpose(1, 2, 0, 3, 4).reshape(1, 10, L, L)


# revision 5
# speedup vs baseline: 1.1477x; 1.0150x over previous
"""Distributed AttentionModel kernel for 8 trn2 NeuronCores.

Strategy: row-shard the 256x256 spatial grid across 8 cores (32 owned rows
per core) with a 17-row halo on each side (= receptive field of the whole
net up to the final conv). Each core recomputes its halo locally (no halo
exchanges); the only cross-core traffic is the per-channel InstanceNorm
statistics (sum / sum-of-squares), reduced with lax.psum. Each core emits
its 32 owned output rows; the host concatenates them.
"""
import numpy as np
import jax
import jax.numpy as jnp

L = 256
NDEV = 8
OWN = L // NDEV          # 32 rows owned per core
HALO = 17                # receptive field: 3 blocks (2*1+2*2+2*4) + f1 + attn + f2
SLAB = OWN + 2 * HALO    # 66 rows per core
DILATIONS = (1, 2, 4)
REGION = 3
EPS = 1e-5

SLAB_STARTS = [min(max(OWN * d - HALO, 0), L - SLAB) for d in range(NDEV)]
OWN_OFFS = [OWN * d - SLAB_STARTS[d] for d in range(NDEV)]


def _conv2d(x, w, b, dilation=1):
    k = w.shape[-1]
    p = dilation * (k - 1) // 2
    y = jax.lax.conv_general_dilated(
        x, w, window_strides=(1, 1), padding=((p, p), (p, p)),
        rhs_dilation=(dilation, dilation),
        dimension_numbers=('NCHW', 'OIHW', 'NCHW'))
    return y + b[None, :, None, None]


def _inorm(x, mask):
    # x: [1,C,S,256]; mask: [S] (1.0 on this core's owned rows).
    # Global mean/var over the full 256x256 image via psum of masked sums.
    xm = x * mask[None, None, :, None]
    s = jax.lax.psum(jnp.sum(xm, axis=(2, 3)), 'i')
    s2 = jax.lax.psum(jnp.sum(xm * x, axis=(2, 3)), 'i')
    n = float(L * L)
    m = s / n
    v = s2 / n - m * m
    return (x - m[:, :, None, None]) / jnp.sqrt(v[:, :, None, None] + EPS)


def _block(h, w1, b1, w2, b2, dilation, mask):
    y = jax.nn.relu(_inorm(_conv2d(h, w1, b1, dilation), mask))
    y = _inorm(_conv2d(y, w2, b2, dilation), mask)
    return jax.nn.relu(y + h)


def _regional_attention(x, wq, wk, wv, wo, bo):
    xh = x[0].transpose(1, 2, 0)            # [S,256,64]
    S, W = xh.shape[0], xh.shape[1]

    q = (xh @ wq.T).reshape(S, W, 4, 16)    # [S,W,h,c]
    k = (xh @ wk.T).reshape(S, W, 4, 16)
    v = (xh @ wv.T).reshape(S, W, 4, 16)

    kp = jnp.pad(k, ((1, 1), (1, 1), (0, 0), (0, 0)))
    vp = jnp.pad(v, ((1, 1), (1, 1), (0, 0), (0, 0)))
    shifts = [(i, j) for i in range(REGION) for j in range(REGION)]

    # logits per neighbor, accumulated without the [S,W,9,16] intermediate
    qs = q / np.sqrt(64.0)
    logits = [jnp.sum(qs * kp[i:i + S, j:j + W], axis=-1) for i, j in shifts]
    qk = jnp.stack(logits, axis=-1)         # [S,W,4,9]
    mx = jnp.max(qk, axis=-1, keepdims=True)
    e = jnp.exp(qk - mx)
    attn = e / jnp.sum(e, axis=-1, keepdims=True)

    out = jnp.zeros((S, W, 4, 16), dtype=x.dtype)
    for n, (i, j) in enumerate(shifts):
        out = out + attn[..., n:n + 1] * vp[i:i + S, j:j + W]
    out = out.reshape(S, W, 64) @ wo.T + bo
    return out.transpose(2, 0, 1)[None]


def _forward(xs, params, mask, off):
    # xs: [441,SLAB,256] slab for this core; mask: [SLAB]; off: [] int32
    h = xs[None]
    h = jax.nn.relu(_inorm(_conv2d(h, params['c1_w'], params['c1_b']), mask))
    for i, dil in enumerate(DILATIONS):
        h = _block(h, params[f'b{i}_w1'], params[f'b{i}_b1'],
                   params[f'b{i}_w2'], params[f'b{i}_b2'], dil, mask)
    h = jax.nn.relu(_inorm(_conv2d(h, params['f1_w'], params['f1_b']), mask))
    h = _regional_attention(h, params['wq'], params['wk'], params['wv'],
                            params['wo'], params['bo'])
    y = _conv2d(h, params['f2_w'], params['f2_b'])   # [1,10,SLAB,256]
    return jax.lax.dynamic_slice_in_dim(y, off, OWN, axis=2)


_pmapped = jax.pmap(_forward, axis_name='i', in_axes=(0, None, 0, 0))


_cache = {}


def kernel(x, params):
    x = np.asarray(x)
    slabs = np.stack([x[0, :, s:s + SLAB, :] for s in SLAB_STARTS])
    if 'masks' not in _cache:
        masks = np.zeros((NDEV, SLAB), dtype=np.float32)
        for d in range(NDEV):
            masks[d, OWN_OFFS[d]:OWN_OFFS[d] + OWN] = 1.0
        _cache['masks'] = jnp.asarray(masks)
        _cache['offs'] = jnp.asarray(np.asarray(OWN_OFFS, dtype=np.int32))
    _cache['params'] = {k: jnp.asarray(v) for k, v in params.items()}
    out = _pmapped(jnp.asarray(slabs), _cache['params'],
                   _cache['masks'], _cache['offs'])
    out = np.asarray(out)                    # [8,1,10,32,256]
    return out.transpose(1, 2, 0, 3, 4).reshape(1, 10, L, L)


# revision 6
# speedup vs baseline: 1.1493x; 1.0013x over previous
"""Distributed AttentionModel kernel for 8 trn2 NeuronCores.

Strategy: row-shard the 256x256 spatial grid across 8 cores (32 owned rows
per core) with a 17-row halo on each side (= receptive field of the whole
net up to the final conv). Each core recomputes its halo locally (no halo
exchanges); the only cross-core traffic is the per-channel InstanceNorm
statistics (sum / sum-of-squares), reduced with lax.psum. Each core emits
its 32 owned output rows; the host concatenates them.
"""
import numpy as np
import jax
import jax.numpy as jnp

L = 256
NDEV = 8
OWN = L // NDEV          # 32 rows owned per core
HALO = 17                # receptive field: 3 blocks (2*1+2*2+2*4) + f1 + attn + f2
SLAB = OWN + 2 * HALO    # 66 rows per core
DILATIONS = (1, 2, 4)
REGION = 3
EPS = 1e-5

SLAB_STARTS = [min(max(OWN * d - HALO, 0), L - SLAB) for d in range(NDEV)]
OWN_OFFS = [OWN * d - SLAB_STARTS[d] for d in range(NDEV)]


def _conv2d(x, w, b, dilation=1):
    k = w.shape[-1]
    if k == 1:
        # 1x1 conv as a dense matmul over flattened pixels (441->64 is the
        # single largest conv; the dense matmul lowers better on neuronx)
        C, S, W2 = x.shape[1], x.shape[2], x.shape[3]
        y = w[:, :, 0, 0] @ x[0].reshape(C, S * W2)
        return y.reshape(1, -1, S, W2) + b[None, :, None, None]
    p = dilation * (k - 1) // 2
    y = jax.lax.conv_general_dilated(
        x, w, window_strides=(1, 1), padding=((p, p), (p, p)),
        rhs_dilation=(dilation, dilation),
        dimension_numbers=('NCHW', 'OIHW', 'NCHW'))
    return y + b[None, :, None, None]


def _inorm(x, mask):
    # x: [1,C,S,256]; mask: [S] (1.0 on this core's owned rows).
    # Global mean/var over the full 256x256 image via psum of masked sums.
    xm = x * mask[None, None, :, None]
    s = jax.lax.psum(jnp.sum(xm, axis=(2, 3)), 'i')
    s2 = jax.lax.psum(jnp.sum(xm * x, axis=(2, 3)), 'i')
    n = float(L * L)
    m = s / n
    v = s2 / n - m * m
    return (x - m[:, :, None, None]) / jnp.sqrt(v[:, :, None, None] + EPS)


def _block(h, w1, b1, w2, b2, dilation, mask):
    y = jax.nn.relu(_inorm(_conv2d(h, w1, b1, dilation), mask))
    y = _inorm(_conv2d(y, w2, b2, dilation), mask)
    return jax.nn.relu(y + h)


def _regional_attention(x, wq, wk, wv, wo, bo):
    xh = x[0].transpose(1, 2, 0)            # [S,256,64]
    S, W = xh.shape[0], xh.shape[1]

    q = (xh @ wq.T).reshape(S, W, 4, 16)    # [S,W,h,c]
    k = (xh @ wk.T).reshape(S, W, 4, 16)
    v = (xh @ wv.T).reshape(S, W, 4, 16)

    kp = jnp.pad(k, ((1, 1), (1, 1), (0, 0), (0, 0)))
    vp = jnp.pad(v, ((1, 1), (1, 1), (0, 0), (0, 0)))
    shifts = [(i, j) for i in range(REGION) for j in range(REGION)]

    # logits per neighbor, accumulated without the [S,W,9,16] intermediate
    qs = q / np.sqrt(64.0)
    logits = [jnp.sum(qs * kp[i:i + S, j:j + W], axis=-1) for i, j in shifts]
    qk = jnp.stack(logits, axis=-1)         # [S,W,4,9]
    mx = jnp.max(qk, axis=-1, keepdims=True)
    e = jnp.exp(qk - mx)
    attn = e / jnp.sum(e, axis=-1, keepdims=True)

    out = jnp.zeros((S, W, 4, 16), dtype=x.dtype)
    for n, (i, j) in enumerate(shifts):
        out = out + attn[..., n:n + 1] * vp[i:i + S, j:j + W]
    out = out.reshape(S, W, 64) @ wo.T + bo
    return out.transpose(2, 0, 1)[None]


def _forward(xs, params, mask, off):
    # xs: [441,SLAB,256] slab for this core; mask: [SLAB]; off: [] int32
    h = xs[None]
    h = jax.nn.relu(_inorm(_conv2d(h, params['c1_w'], params['c1_b']), mask))
    for i, dil in enumerate(DILATIONS):
        h = _block(h, params[f'b{i}_w1'], params[f'b{i}_b1'],
                   params[f'b{i}_w2'], params[f'b{i}_b2'], dil, mask)
    h = jax.nn.relu(_inorm(_conv2d(h, params['f1_w'], params['f1_b']), mask))
    h = _regional_attention(h, params['wq'], params['wk'], params['wv'],
                            params['wo'], params['bo'])
    y = _conv2d(h, params['f2_w'], params['f2_b'])   # [1,10,SLAB,256]
    return jax.lax.dynamic_slice_in_dim(y, off, OWN, axis=2)


_pmapped = jax.pmap(_forward, axis_name='i', in_axes=(0, None, 0, 0))


_cache = {}


def kernel(x, params):
    x = np.asarray(x)
    slabs = np.stack([x[0, :, s:s + SLAB, :] for s in SLAB_STARTS])
    if 'masks' not in _cache:
        masks = np.zeros((NDEV, SLAB), dtype=np.float32)
        for d in range(NDEV):
            masks[d, OWN_OFFS[d]:OWN_OFFS[d] + OWN] = 1.0
        _cache['masks'] = jnp.asarray(masks)
        _cache['offs'] = jnp.asarray(np.asarray(OWN_OFFS, dtype=np.int32))
    _cache['params'] = {k: jnp.asarray(v) for k, v in params.items()}
    out = _pmapped(jnp.asarray(slabs), _cache['params'],
                   _cache['masks'], _cache['offs'])
    out = np.asarray(out)                    # [8,1,10,32,256]
    return out.transpose(1, 2, 0, 3, 4).reshape(1, 10, L, L)
